# revision 91
# baseline (speedup 1.0000x reference)
# Causal self-attention (B=2, T=4096, C=768, H=12, D=64) on 8 trn2 cores.
#
# Sharding: core c = 4*b + hg handles batch b and head-group hg (3 heads).
# Per core:
#   qT,kT (d-major) and v (t-major) via QKV matmuls against DMA-transposed x
#   causal attention: scores in scoresT orientation (tk partitions, tq free),
#   exp on ACT, then PV in y-orientation: y[q,d] accumulated via 65-wide
#   matmuls (lhsT = exp-scores q-slices, rhs = v plus a ones column whose
#   output column is the softmax denominator). Normalization is a per-
#   partition reciprocal multiply, then PE transposes y back to d-major for
#   the projection, which packs heads 0+1 into a single K=128 chain.
#   Host sums the 4 head-group partials per batch.
import numpy as np

B, T, C = 2, 4096, 768
H, D = 12, 64
NHL = 3          # heads per core
CT = C // 128    # 6 contraction tiles

_PROG_CACHE = {}

# test-harness hooks (harmless when unused): set TRACE=True before calling
# kernel() to capture an NTFF profile; the BassKernelResults lands in LAST.
TRACE = False
LAST = None


def _build(t_len, has_ba):
    import concourse.bass as bass
    import concourse.bacc as bacc
    import concourse.mybir as mybir
    import concourse.tile as tile
    from concourse.bass import ts, ds

    f32 = mybir.dt.float32
    bf16 = mybir.dt.bfloat16
    AF = mybir.ActivationFunctionType
    ALU = mybir.AluOpType

    TB = t_len // 128   # 128-row t tiles
    QB = t_len // 512   # 512-row q blocks

    nc = bacc.Bacc("TRN2", target_bir_lowering=False, debug=False)

    x_d = nc.dram_tensor("x", [t_len, C], f32, kind="ExternalInput").ap()
    wqk_d = nc.dram_tensor("wqk", [C, 384], f32, kind="ExternalInput").ap()
    wv_d = nc.dram_tensor("wv", [C, NHL * D], f32, kind="ExternalInput").ap()
    wp_d = nc.dram_tensor("wp", [NHL * D, C], f32, kind="ExternalInput").ap()
    if has_ba:
        baqk_d = nc.dram_tensor("baqk", [1, 384], f32, kind="ExternalInput").ap()
        bav_d = nc.dram_tensor("bav", [1, NHL * D], f32, kind="ExternalInput").ap()
    out_d = nc.dram_tensor("out", [t_len, C], bf16, kind="ExternalOutput").ap()

    with tile.TileContext(nc) as tc:
        with (
            tc.tile_pool(name="const", bufs=1) as constp,
            tc.tile_pool(name="big", bufs=1) as bigp,
            tc.tile_pool(name="xload", bufs=2) as xloadp,
            tc.tile_pool(name="xl32", bufs=2) as xl32p,
            tc.tile_pool(name="xTp", bufs=2) as xTp,
            tc.tile_pool(name="expp", bufs=12) as expp,
            tc.tile_pool(name="ytsp", bufs=12) as ytsp,
            tc.tile_pool(name="yTp", bufs=4) as yTp,
            tc.tile_pool(name="accp", bufs=8) as accp,
            tc.tile_pool(name="small", bufs=12) as smallp,
            tc.tile_pool(name="psA", bufs=2, space="PSUM") as psA,
            tc.tile_pool(name="psY", bufs=1, space="PSUM") as psY,
            tc.tile_pool(name="psP", bufs=2, space="PSUM") as psP,
        ):
            # ---- persistent SBUF tensors ----
            qkT = bigp.tile([128, 4, t_len], bf16)        # [q0|q1],[k0|k1],[q2|k2],[k2|-]
            vaug = bigp.tile([128, TB, NHL * 65], bf16)   # v + ones col per head
            wqk_sb = bigp.tile([128, CT, 384], bf16)
            wv_sb = bigp.tile([128, CT, NHL * D], bf16)
            wp01_sb = bigp.tile([128, C], bf16)
            wp2_sb = bigp.tile([64, C], bf16)
            masks_sb = bigp.tile([128, 2048], bf16)
            maskshh_sb = bigp.tile([128, 4096], bf16)
            ident_sb = constp.tile([128, 128], bf16)
            if has_ba:
                baqk_sb = constp.tile([1, 384], bf16)
                bav_sb = constp.tile([1, NHL * D], bf16)
                ones128 = constp.tile([1, 128], bf16)
                ones512 = constp.tile([1, 512], bf16)
                nc.vector.memset(ones128, 1.0)
                nc.vector.memset(ones512, 1.0)
                nc.gpsimd.dma_start(out=baqk_sb, in_=baqk_d)
                nc.gpsimd.dma_start(out=bav_sb, in_=bav_d)

            # ones columns of vaug (col 64 of each head's 65-wide chunk)
            vaug4 = vaug.rearrange("p t (h e) -> p t h e", e=65)
            for h in range(NHL):
                nc.vector.memset(vaug4[:, :, h, 64:65], 1.0)

            # scores operand slices: head -> (partition offset, qk tile indices)
            def q_ap(h, J):
                p0, nt = [(0, 0), (64, 0), (0, 2)][h]
                return qkT[p0 : p0 + 64, nt, ts(J, 512)]

            def k_ap(h, kt):
                p0, nt = [(0, 1), (64, 1), (0, 3)][h]
                return qkT[p0 : p0 + 64, nt, ts(kt, 128)]

            def load_x_dma(J):
                """Block J's casting x load (SWDGE). Emitted two blocks
                ahead; the transposes are emitted separately/later so they
                never hold the in-order sync queue while waiting on it."""
                xbf = xloadp.tile([128, 4, C], bf16, tag="xbf", name=f"xb{J % 2}")
                nc.gpsimd.dma_start(
                    out=xbf,
                    in_=x_d[ts(J, 512), :].rearrange("(a p) c -> p a c", p=128),
                )
                return xbf

            def load_x_tr(J, xbf):
                xTb = xTp.tile([128, CT, 512], bf16, tag="xT", name=f"xT{J % 2}")
                for sub in range(4):
                    nc.sync.dma_start(
                        out=xTb[:, :, ts(sub, 128)],
                        in_=xbf[:, sub, :],
                        transpose=True,
                    )
                return xTb

            def load_x(J):
                return load_x_tr(J, load_x_dma(J))

            def qkv_groups(J, xTb):
                """Fine-grained closures (one matmul or evac each) over a
                loaded xTb, interleaved into the score loops so the PE work
                arrives in ~200ns quanta instead of 1.3us chain lumps."""
                st = {}

                def qk_mm(nt, ct):
                    if ct == 0:
                        st[nt] = psP.tile(
                            [128, 512], f32, tag="pj", name="qkps"
                        )
                    nc.tensor.matmul(
                        st[nt],
                        wqk_sb[:, ct, ts(nt, 128)],
                        xTb[:, ct, :],
                        start=(ct == 0),
                        stop=(ct == CT - 1 and not has_ba),
                    )

                def qk_ev(nt):
                    if has_ba:
                        nc.tensor.matmul(
                            st[nt], baqk_sb[:, ts(nt, 128)], ones512,
                            start=False, stop=True,
                        )
                    nc.vector.tensor_copy(
                        out=qkT[:, nt, ts(J, 512)], in_=st[nt]
                    )

                def v_mm(sub, ct):
                    if ct == 0:
                        st[4 + sub] = psP.tile(
                            [128, NHL * D], f32, tag="pj", name="vps"
                        )
                    nc.tensor.matmul(
                        st[4 + sub],
                        xTb[:, ct, ts(sub, 128)],
                        wv_sb[:, ct, :],
                        start=(ct == 0),
                        stop=(ct == CT - 1 and not has_ba),
                    )

                def v_ev(sub):
                    if has_ba:
                        nc.tensor.matmul(
                            st[4 + sub], ones128, bav_sb, start=False, stop=True
                        )
                    nc.vector.tensor_copy(
                        out=vaug4[:, J * 4 + sub, :, 0:64],
                        in_=st[4 + sub].rearrange("p (h e) -> p h e", e=64),
                    )

                def k2_shift():
                    # nt3[0:64] = k2 (from nt2's upper half) so h2's scores
                    # get base-aligned operands without a 4th matmul group
                    nc.gpsimd.dma_start(
                        out=qkT[0:64, 3, ts(J, 512)],
                        in_=qkT[64:128, 2, ts(J, 512)],
                    )

                def group(kind, idx):
                    if kind == "qk":
                        return [
                            lambda ct=ct: qk_mm(idx, ct) for ct in range(CT)
                        ] + [lambda: qk_ev(idx)]
                    return [
                        lambda ct=ct: v_mm(idx, ct) for ct in range(CT)
                    ] + [lambda: v_ev(idx)]

                # qk tiles 0,1 first (h0/h1 scores depend on them), then v
                # (PV), then qk 2 + the k2 partition-shift (h2 comes last)
                return (
                    group("qk", 0) + group("qk", 1)
                    + group("v", 0) + group("v", 1)
                    + group("v", 2) + group("v", 3)
                    + group("qk", 2) + [k2_shift]
                )

            def koff(Jq, kt):
                # first valid q column for (possibly diagonal) ktile kt
                return 128 * (kt - 4 * Jq) if kt >= 4 * Jq else 0

            def scores_exp_h01(Jq, kt):
                    J = Jq
                    off = koff(Jq, kt)
                    sc = psA.tile([128, 1024], f32, tag="psa")
                    nc.tensor.matmul(
                        sc[:, off:512], k_ap(0, kt), q_ap(0, J)[:, off:512],
                        start=True, stop=True,
                    )
                    nc.tensor.matmul(
                        sc[:, 512 + off : 1024], k_ap(1, kt),
                        q_ap(1, J)[:, off:512],
                        start=True, stop=True,
                    )
                    ex = expp.tile([128, 1024], bf16, tag="ex")
                    sc2 = sc.rearrange("p (two n) -> p two n", two=2)
                    ex2 = ex.rearrange("p (two n) -> p two n", two=2)
                    nc.scalar.activation(
                        ex2[:, :, off:512], sc2[:, :, off:512], AF.Exp, scale=0.125
                    )
                    if kt >= 4 * J:
                        m4 = maskshh_sb.rearrange(
                            "p (j two n) -> p j two n", j=4, two=2
                        )
                        nc.vector.tensor_mul(
                            ex2[:, :, off:512],
                            ex2[:, :, off:512],
                            m4[:, kt - 4 * J, :, off:512],
                        )
                    return ex

            def scores_exp_h2(Jq, g):
                    J = Jq
                    offa, offb = koff(Jq, 2 * g), koff(Jq, 2 * g + 1)
                    sc = psA.tile([128, 1024], f32, tag="psa")
                    nc.tensor.matmul(
                        sc[:, offa:512], k_ap(2, 2 * g), q_ap(2, J)[:, offa:512],
                        start=True, stop=True,
                    )
                    nc.tensor.matmul(
                        sc[:, 512 + offb : 1024], k_ap(2, 2 * g + 1),
                        q_ap(2, J)[:, offb:512],
                        start=True, stop=True,
                    )
                    ex = expp.tile([128, 1024], bf16, tag="ex")
                    if offa == offb:
                        sc2 = sc.rearrange("p (two n) -> p two n", two=2)
                        ex2 = ex.rearrange("p (two n) -> p two n", two=2)
                        nc.scalar.activation(
                            ex2[:, :, offa:512], sc2[:, :, offa:512],
                            AF.Exp, scale=0.125,
                        )
                    else:
                        nc.scalar.activation(
                            ex[:, offa:512], sc[:, offa:512], AF.Exp, scale=0.125
                        )
                        nc.scalar.activation(
                            ex[:, 512 + offb : 1024], sc[:, 512 + offb : 1024],
                            AF.Exp, scale=0.125,
                        )
                    if g >= 2 * Jq:
                        ja, jb = 2 * g - 4 * Jq, 2 * g + 1 - 4 * Jq
                        m4 = masks_sb.rearrange("p (j n) -> p j n", j=4)
                        nc.vector.tensor_mul(
                            ex[:, offa:512],
                            ex[:, offa:512],
                            m4[:, ja, offa:512],
                        )
                        nc.vector.tensor_mul(
                            ex[:, 512 + offb : 1024],
                            ex[:, 512 + offb : 1024],
                            m4[:, jb, offb:512],
                        )
                    return ex

            # ---- pipelined J loop: a shared work queue of deferred QKV and
            # projection matmul groups is drained across both score loops.
            # The DMA device is serial and serves by readiness, so the only
            # ordering control is same-queue position: the critical x(0),
            # wqk, wv go first; x(1..) and wp ride the sync queue behind the
            # block-0 transposes ----
            xT0 = load_x(0)
            nc.gpsimd.dma_start(
                out=wqk_sb, in_=wqk_d.rearrange("(ct p) n -> p ct n", p=128)
            )
            nc.gpsimd.dma_start(
                out=wv_sb, in_=wv_d.rearrange("(ct p) n -> p ct n", p=128)
            )
            xT1 = load_x(1) if QB > 1 else None
            nc.gpsimd.dma_start(out=wp01_sb, in_=wp_d[0:128, :])
            nc.gpsimd.dma_start(out=wp2_sb, in_=wp_d[128:192, :])
            # masks and the transpose identity are generated on-device (DVE
            # is idle at startup; this keeps the serial DMA device free for
            # the x/weight loads): mask_j[p, f] = (f - p - 128j >= 0)
            m4g = masks_sb.rearrange("p (j n) -> p j n", j=4)
            mh4g = maskshh_sb.rearrange("p (j two n) -> p j two n", j=4, two=2)
            nc.gpsimd.memset(masks_sb, 1.0)
            nc.gpsimd.memset(maskshh_sb, 1.0)
            nc.gpsimd.memset(ident_sb, 1.0)
            for j in range(4):
                nc.gpsimd.affine_select(
                    out=m4g[:, j, :], in_=m4g[:, j, :],
                    compare_op=mybir.AluOpType.is_ge, fill=0.0,
                    base=-128 * j, channel_multiplier=-1,
                    pattern=[[1, 512]],
                )
                nc.gpsimd.affine_select(
                    out=mh4g[:, j, :, :], in_=mh4g[:, j, :, :],
                    compare_op=mybir.AluOpType.is_ge, fill=0.0,
                    base=-128 * j, channel_multiplier=-1,
                    pattern=[[0, 2], [1, 512]],
                )
            nc.gpsimd.affine_select(
                out=ident_sb, in_=ident_sb,
                compare_op=mybir.AluOpType.is_equal, fill=0.0,
                base=0, channel_multiplier=1,
                pattern=[[-1, 128]],
            )

            # the two qk groups h0/h1's first scores need run up front; the
            # rest of block 0 drains inside J=0's loops (before PV uses it)
            groups0 = qkv_groups(0, xT0)
            for part in groups0[: 2 * (CT + 1)]:
                part()

            pending_work = groups0[2 * (CT + 1) :] + (
                qkv_groups(1, xT1) if QB > 1 else []
            )
            xT_next = None
            pre_exs = []
            PRE_MAX = 1

            for J in range(QB):
                nkt = 4 * (J + 1)
                npair = nkt // 2
                # y accumulators: per bank, [q=128, (qslice 2, head 3, 65)]
                # col 64 of each 65-chunk = softmax denominator (ones column)
                yA = psY.tile([128, 2 * NHL * 65, ], f32, tag="ya")
                yB = psY.tile([128, 2 * NHL * 65, ], f32, tag="yb")
                yA4 = yA.rearrange("p (s h e) -> p s h e", h=NHL, e=65)
                yB4 = yB.rearrange("p (s h e) -> p s h e", h=NHL, e=65)
                bank_started = {"A": False, "B": False}

                def y_slot(i):
                    if i < 2:
                        return yA4, i, "A"
                    return yB4, i - 2, "B"

                def pv_mm(h, kt, i, ex_col0, ex):
                    Y4, s, bank = y_slot(i)
                    st = not bank_started[bank]
                    bank_started[bank] = True
                    stop = h == 2 and (
                        (i == 1 and kt == 4 * J + 1)
                        or (i == 3 and kt == 4 * J + 3)
                    )
                    nc.tensor.matmul(
                        Y4[:, s, h, 0:65],
                        ex[:, ex_col0 + 128 * i : ex_col0 + 128 * i + 128],
                        vaug4[:, kt, h, 0:65],
                        start=st, stop=stop,
                    )

                def pv_h01(kt, ex):
                    i_min = max(0, kt - 4 * J)
                    for i in range(i_min, 4):
                        pv_mm(0, kt, i, 0, ex)
                        pv_mm(1, kt, i, 512, ex)

                def pv_h2(g, ex):
                    for sub in range(2):
                        kt = 2 * g + sub
                        i_min = max(0, kt - 4 * J)
                        for i in range(i_min, 4):
                            pv_mm(2, kt, i, 512 * sub, ex)

                # two-block QKV lookahead: x(J+2)'s DMA issues at J start, but
                # only J+1's matmul groups (data loaded during J-1) weave into
                # the score loops — never put a matmul in the in-order PE
                # queue before its DMA data can be resident. J+1's qk groups
                # lead so its pre-emitted scores (below) have operands.
                work = pending_work
                if xT_next is not None:
                    g1 = qkv_groups(J + 1, xT_next)
                    work = g1[: 2 * (CT + 1)] + work + g1[2 * (CT + 1) :]
                xT_next = load_x(J + 2) if J + 2 < QB else None
                pending_work = []
                wu, wi = 0, 0
                total_units = max(2 * (nkt - 1) + (npair - 1), 1)

                def drain():
                    nonlocal wi
                    want = wu * len(work) // total_units
                    while wi < want:
                        work[wi]()
                        wi += 1

                npre = len(pre_exs)
                ex_p = pre_exs[0] if npre else scores_exp_h01(J, 0)
                for kt in range(1, nkt):
                    ex = (
                        pre_exs[kt] if kt < npre else scores_exp_h01(J, kt)
                    )
                    wu += 1
                    drain()
                    pv_h01(kt - 1, ex_p)
                    ex_p = ex
                pv_h01(nkt - 1, ex_p)

                # the h2 loop is PE-heavy (2 ktiles per exp tile), so weave
                # J+1's first h01 score/exp tiles in to keep ACT saturated
                # across the block boundary; their PV runs next iteration
                pre_exs = []

                def pre_emit():
                    if J + 1 < QB and len(pre_exs) < PRE_MAX:
                        pre_exs.append(
                            scores_exp_h01(J + 1, len(pre_exs))
                        )

                ex_p = scores_exp_h2(J, 0)
                for g in range(1, npair):
                    ex = scores_exp_h2(J, g)
                    wu += 1
                    drain()
                    pv_h2(g - 1, ex_p)
                    ex_p = ex
                pv_h2(npair - 1, ex_p)
                pre_emit()
                while wi < len(work):
                    work[wi]()
                    wi += 1
                # ---- finalize: reciprocal of sums, normalize, transpose ----
                rcpA = smallp.tile([128, 2 * NHL], f32, tag="rcpA")
                rcpB = smallp.tile([128, 2 * NHL], f32, tag="rcpB")
                yAs = yA.rearrange("p (sh e) -> p sh e", e=65)
                yBs = yB.rearrange("p (sh e) -> p sh e", e=65)
                nc.vector.reciprocal_approx_fast(rcpA, yAs[:, :, 64:65])
                nc.vector.reciprocal_approx_fast(rcpB, yBs[:, :, 64:65])
                yts_l = []
                for i in range(4):
                    Y4, s, bank = y_slot(i)
                    rcp = rcpA if bank == "A" else rcpB
                    yts = ytsp.tile([128, NHL, D], bf16, tag="yts", name=f"yts{i}")
                    rcp3 = rcp.rearrange("p (s h) -> p s h", h=NHL)[
                        :, s, :, None
                    ].broadcast_to([128, NHL, D])
                    nc.vector.tensor_tensor(
                        out=yts, in0=Y4[:, s, :, 0:64], in1=rcp3, op=ALU.mult
                    )
                    yts_l.append(yts)

                if J == QB - 1:
                    # tail: per-q-slice pipeline so output row-block i drains
                    # as soon as slice i is normalized, instead of waiting
                    # for the full per-head transpose tiles
                    for i in range(4):
                        tq = psP.tile(
                            [64, NHL * 128], bf16, tag="pj", name=f"tq{i}"
                        )
                        for h in range(NHL):
                            nc.tensor.transpose(
                                tq[:, ts(h, 128)], yts_l[i][:, h, :], ident_sb
                            )
                        yTq = ytsp.tile(
                            [128, 128], bf16, tag="yTq", name=f"yq{i}"
                        )
                        yTq2 = ytsp.tile(
                            [64, 128], bf16, tag="yTq2", name=f"yq2{i}"
                        )
                        nc.vector.tensor_copy(
                            out=yTq[0:64, :], in_=tq[:, 0:128]
                        )
                        nc.vector.tensor_copy(
                            out=yTq[64:128, :], in_=tq[:, 128:256]
                        )
                        nc.vector.tensor_copy(out=yTq2, in_=tq[:, 256:384])
                        acc = accp.tile([128, C], bf16, tag="acc", name="acc")
                        for half in range(2):
                            wid = 512 if half == 0 else 256
                            c0 = 512 * half
                            pj = psA.tile(
                                [128, wid], f32, tag="psa", name=f"pjt{half}"
                            )
                            nc.tensor.matmul(
                                pj, yTq, wp01_sb[:, c0 : c0 + wid],
                                start=True, stop=False,
                            )
                            nc.tensor.matmul(
                                pj, yTq2, wp2_sb[:, c0 : c0 + wid],
                                start=False, stop=True,
                            )
                            nc.vector.tensor_copy(
                                out=acc[:, c0 : c0 + wid], in_=pj
                            )
                        nc.sync.dma_start(
                            out=out_d[ds(J * 512 + i * 128, 128), :], in_=acc
                        )
                    for part in pending_work:
                        part()
                    pending_work = []
                    break
                yT01 = yTp.tile([128, 512], bf16, tag="y01")
                yT2 = yTp.tile([64, 512], bf16, tag="y2")
                for h in range(NHL):
                    trp = psP.tile([64, 512], bf16, tag="pj", name=f"trp{h}")
                    for i in range(4):
                        nc.tensor.transpose(
                            trp[:, ts(i, 128)], yts_l[i][:, h, :], ident_sb
                        )
                    if h < 2:
                        nc.vector.tensor_copy(
                            out=yT01[ds(64 * h, 64), :], in_=trp
                        )
                    else:
                        nc.vector.tensor_copy(out=yT2, in_=trp)

                def proj_parts(Jp, yT01p, yT2p):
                    """Fine-grained closures (one matmul or evac each), one
                    128-row output tile per jj. The last block's projection
                    uses the (by then idle) scores pool so its matmuls and
                    evac copies double-buffer."""
                    pool, tag = (psA, "psa") if Jp == QB - 1 else (psP, "pj")
                    pst = {}

                    def mm(jj, half, part):
                        wid = 512 if half == 0 else 256
                        c0 = 512 * half
                        if part == 0:
                            pst[0] = pool.tile(
                                [128, wid], f32, tag=tag, name=f"pj{half}"
                            )
                            nc.tensor.matmul(
                                pst[0], yT01p[:, ts(jj, 128)],
                                wp01_sb[:, c0 : c0 + wid],
                                start=True, stop=False,
                            )
                        else:
                            nc.tensor.matmul(
                                pst[0], yT2p[:, ts(jj, 128)],
                                wp2_sb[:, c0 : c0 + wid],
                                start=False, stop=True,
                            )

                    def ev(jj, half):
                        c0 = 512 * half
                        wid = 512 if half == 0 else 256
                        if half == 0:
                            pst[1] = accp.tile([128, C], bf16, tag="acc", name="acc")
                        nc.vector.tensor_copy(
                            out=pst[1][:, c0 : c0 + wid], in_=pst[0]
                        )
                        if half == 1:
                            nc.sync.dma_start(
                                out=out_d[ds(Jp * 512 + jj * 128, 128), :],
                                in_=pst[1],
                            )

                    out_parts = []
                    for jj in range(4):
                        for half in range(2):
                            out_parts.append(
                                lambda jj=jj, half=half: mm(jj, half, 0)
                            )
                            out_parts.append(
                                lambda jj=jj, half=half: mm(jj, half, 1)
                            )
                            out_parts.append(
                                lambda jj=jj, half=half: ev(jj, half)
                            )
                    return out_parts

                pending_work = proj_parts(J, yT01, yT2) + pending_work
            for part in pending_work:
                part()
    nc.compile()
    return nc


def _core_inputs(x, w_attn, b_attn, hg, t_len, has_ba):
    # column ranges of this head-group inside w_attn: q | k | v blocks of C each
    q0 = 192 * hg
    wqk = np.zeros((C, 3, 128), np.float32)
    wqk[:, 0, 0:64] = w_attn[:, q0 : q0 + 64]                 # q0
    wqk[:, 0, 64:128] = w_attn[:, q0 + 64 : q0 + 128]         # q1
    wqk[:, 1, 0:64] = w_attn[:, C + q0 : C + q0 + 64]         # k0
    wqk[:, 1, 64:128] = w_attn[:, C + q0 + 64 : C + q0 + 128] # k1
    wqk[:, 2, 0:64] = w_attn[:, q0 + 128 : q0 + 192]          # q2
    wqk[:, 2, 64:128] = w_attn[:, C + q0 + 128 : C + q0 + 192]  # k2
    ins = {
        "x": np.ascontiguousarray(x),
        "wqk": np.ascontiguousarray(wqk.reshape(C, 384)),
        "wv": np.ascontiguousarray(w_attn[:, 2 * C + q0 : 2 * C + q0 + 192]),
    }
    if has_ba:
        baqk = np.zeros((1, 384), np.float32)
        baqk[0, 0:64] = b_attn[q0 : q0 + 64]
        baqk[0, 64:128] = b_attn[q0 + 64 : q0 + 128]
        baqk[0, 128:192] = b_attn[C + q0 : C + q0 + 64]
        baqk[0, 192:256] = b_attn[C + q0 + 64 : C + q0 + 128]
        baqk[0, 256:320] = b_attn[q0 + 128 : q0 + 192]
        baqk[0, 320:384] = b_attn[C + q0 + 128 : C + q0 + 192]
        ins["baqk"] = baqk
        ins["bav"] = np.ascontiguousarray(
            b_attn[2 * C + q0 : 2 * C + q0 + 192].reshape(1, 192)
        )
    return ins


def kernel(**inputs):
    from concourse.bass_utils import run_bass_kernel_spmd

    x = np.asarray(inputs["x"], dtype=np.float32)
    w_attn = np.asarray(inputs["w_attn"], dtype=np.float32)
    b_attn = np.asarray(inputs["b_attn"], dtype=np.float32)
    w_proj = np.asarray(inputs["w_proj"], dtype=np.float32)
    b_proj = np.asarray(inputs["b_proj"], dtype=np.float32)

    has_ba = bool(np.any(b_attn))
    key = (T, has_ba)
    if key not in _PROG_CACHE:
        _PROG_CACHE[key] = _build(T, has_ba)
    nc = _PROG_CACHE[key]

    in_maps = []
    for c in range(8):
        b, hg = c // 4, c % 4
        ins = _core_inputs(x[b], w_attn, b_attn, hg, T, has_ba)
        ins["wp"] = np.ascontiguousarray(w_proj[192 * hg : 192 * hg + 192, :])
        in_maps.append(ins)

    res = run_bass_kernel_spmd(nc, in_maps, core_ids=list(range(8)), trace=TRACE)
    global LAST
    LAST = res
    out = np.zeros((B, T, C), np.float32)
    for c in range(8):
        out[c // 4] += np.asarray(res.results[c]["out"], dtype=np.float32)
    out += b_proj
    return out
